# revision 10
# baseline (speedup 1.0000x reference)
"""GRU layer kernel for Trainium2, data-parallel over 8 NeuronCores.

Strategy (feature-major / weight-stationary, mixed fp16 + fp8 precision):
  - Shard batch N=2048 -> 8 cores x NB=256. Per-core steady state is
    PE-peak-bound: every 256-col matmul instruction costs ~109 ns
    regardless of dtype; fp8e4m3 DoubleRow packs K=256 per instruction,
    so each part moved to fp8 halves its instruction count (2x math).
  - Precision assignment (error budget 2e-2, sim-validated):
      r gate: fully fp8-DR (error damped by dsigmoid*Whh*(1-z)).
      z gate: x-part fp16; h-part HALF fp8 (contraction features 0..255
        via DR on h8, features 256..511 fp16 on h). x-part in fp8 would
        be 5e-2 (x ~ N(0,1) is large); the h-part tolerates it because
        |h|<1 bounds the operand variance. Full-h-part fp8 is 1.8e-2 --
        too close to the gate; half is 1.3e-2 with both g+z halves 1.75e-2.
      g gate: x-part fp16; h-part half fp8 (features 0..255 via DR on
        rh8a = fp8(r0*h0), features 256..511 fp16 on rh1 = r1*h1). The
        fp8 half is the LOW half because the r0 sigmoid (bank 0) lands
        ~0.9us before r1; rh8a is one fused (r*128)*h scalar_tensor_tensor
        on the DVE, so the Whh DR matmuls can start early while rh1 for
        the fp16 half follows. z and g each use one fused ACT over a
        2-bank PSUM tile (the Pool engine rejects TensorScalar ops at the
        ISA level, so casts stay on ACT/DVE). This keeps the serial
        recurrence chain (r -> rh -> Whh -> tanh -> blend -> h8) under
        the ~7.9us/step PE budget.
  - Scale unification: fp8 operands are pre-scaled so every fp8 product
    is 2^16; the fp16 weights of the z/g gates are pre-scaled by 2^16
    (exact, fits fp16 range) so mixed fp8+fp16 accumulation into one
    PSUM bank descales with a single activation scale=2^-16.
  - h is re-quantized to fp8 fresh each step (no error compounding).
    Intermediates fp16 (DVE 2x mode).
  - DMA: x / x8 / out DRAM tensors are pre-swizzled on the host to
    [T, 128, ...] partition-major contiguous layouts (1-2KB descriptors).
    Startup weight DMAs are partition-chunked (4KB descriptors, 4 queues
    each) and tiered in first-use order, each tier gated on the previous,
    so t=0 starts after ~wzx+x0 land rather than after all weights.
  - Per timestep 6 PSUM banks (2 m-tiles per bank): z, r, g pre-acts;
    sigmoid/tanh on ACT (bank-fused, zero-bias path); blend on DVE.
    The tile/walrus scheduler hoists the next step's x-projections into
    the recurrence latency automatically.
Measured: 584 us baseline -> this version targets ~520 us; max rel err
sim 1.75e-2 (HW historically ~8% better than sim; gate is 2e-2).
"""
import os
import numpy as np

N, D = 2048, 512
T = int(os.environ.get("GRU_T", "64"))
NC = 8
NB = N // NC          # 256 batch rows per core
KT = D // 128         # 4 k-tiles
MT = D // 128         # 4 m-tiles

MM_DT = os.environ.get("GRU_MM_DT", "fp16")   # fp16 | bf16 | fp32r
R8 = bool(int(os.environ.get("GRU_R8", "1")))  # r-gate matmuls in fp8 DoubleRow
H8HALF = bool(int(os.environ.get("GRU_H8HALF", "1"))) and R8  # zh/gh half-fp8

# fp8 scaling: every fp8 product is (W*SW)@(op*S) = 2^16 * W@op; fp16 parts
# of the same PSUM bank are pre-scaled 2^16 on the weight side; all gates
# descale inside the activation via scale=2^-16.
SX, SH = 16.0, 128.0
SWX, SWH = 4096.0, 512.0
PROD = SWX * SX  # == SWH * SH == 65536

_CACHE = {}
LAST_RESULT = None


def _build_nc(zero_bias):
    import concourse.bacc as bacc
    import concourse.mybir as mybir
    from concourse.tile import TileContext

    f32 = mybir.dt.float32
    f8 = mybir.dt.float8e4
    DR = mybir.MatmulPerfMode.DoubleRow
    Mult = mybir.AluOpType.mult
    mdt = {"fp16": mybir.dt.float16, "bf16": mybir.dt.bfloat16,
           "fp32r": mybir.dt.float32r}[MM_DT]
    Sig = mybir.ActivationFunctionType.Sigmoid
    Tanh = mybir.ActivationFunctionType.Tanh
    Copy = mybir.ActivationFunctionType.Copy

    nc = bacc.Bacc("TRN2", target_bir_lowering=False, debug=False, num_devices=NC)

    # x / x8 / out swizzled to [T, 128, ...]: contiguous per partition row
    xt_d = nc.dram_tensor("xt", [T, 128, KT, NB], mdt, kind="ExternalInput")
    w_d = {}
    for wname in ("wzx", "whx"):
        w_d[wname] = nc.dram_tensor(wname, [128, KT * MT * 128], mdt, kind="ExternalInput")
    HKT = 2 if H8HALF else KT  # fp16 k-tiles kept for wzh/whh
    for wname in ("wzh", "whh"):
        w_d[wname] = nc.dram_tensor(wname, [128, HKT * MT * 128], mdt, kind="ExternalInput")
    if not R8:
        for wname in ("wrx", "wrh"):
            w_d[wname] = nc.dram_tensor(wname, [128, KT * MT * 128], mdt, kind="ExternalInput")
    if R8:
        # DoubleRow-packed fp8 weights: w[p, k2, mi, i, m] =
        # W[mi*128+m, (2*k2+i)*128+p] * scale   (k2 absent for the halves)
        xt8_d = nc.dram_tensor("xt8", [T, 128, 2, 2, NB], f8, kind="ExternalInput")
        w8_d = {w: nc.dram_tensor(w, [128, 2 * MT * 2 * 128], f8, kind="ExternalInput")
                for w in ("wrx8", "wrh8")}
        if H8HALF:
            for w in ("wzh8", "whh8"):
                w8_d[w] = nc.dram_tensor(w, [128, MT * 2 * 128], f8, kind="ExternalInput")
    b_d = {}
    for bname in ("bz", "br", "bh"):
        b_d[bname] = nc.dram_tensor(bname, [128, MT], f32, kind="ExternalInput")
    out_d = nc.dram_tensor("out", [T, 128, 2, 2, NB], mdt, kind="ExternalOutput")

    zg_sc = (1.0 / PROD) if R8 else 1.0   # z/g pre-acts carry 2^16
    r_sc = (1.0 / PROD) if R8 else 1.0

    with TileContext(nc) as tc:
        with (
            tc.tile_pool(name="wsb", bufs=1) as wsb,
            tc.tile_pool(name="xsb", bufs=4) as xsb,
            tc.tile_pool(name="ssb", bufs=2) as ssb,
            tc.tile_pool(name="hsb", bufs=3) as hsb,
            tc.tile_pool(name="psum", bufs=1, space="PSUM") as psum,
        ):
            w_sb = {}
            for wname, d in w_d.items():
                w_sb[wname] = wsb.tile(list(d.shape), mdt, name=f"w_{wname}")
            w8_sb = {}
            if R8:
                for w in ("wrx8", "wrh8"):
                    w8_sb[w] = wsb.tile([128, 2 * MT, 2, 128], f8, name=f"w_{w}")
                if H8HALF:
                    for w in ("wzh8", "whh8"):
                        w8_sb[w] = wsb.tile([128, MT, 2, 128], f8, name=f"w_{w}")
            b_sb = {}
            for bname in b_d:
                b_sb[bname] = wsb.tile([128, MT], f32, name=f"b_{bname}")

            from concourse.tile import add_dep_helper

            def pdma(dst, src, pch=4):
                # partition-chunked: big contiguous descriptors, spread
                # across DMA queues
                insts = []
                step = 128 // pch
                for u in range(pch):
                    insts.append(nc.sync.dma_start(
                        out=dst[u * step:(u + 1) * step],
                        in_=src[u * step:(u + 1) * step]))
                return insts

            def gate(insts, on):
                for li in insts:
                    for pi in on:
                        add_dep_helper(li.ins, pi.ins, sync=True,
                                       reason="startup DMA priority")

            def w8dma(w):
                a = w8_d[w].shape[1] // (2 * 128)
                return [nc.sync.dma_start(
                    out=w8_sb[w][:],
                    in_=w8_d[w][:].rearrange("p (a i m) -> p a i m", a=a, i=2))]

            # Tier 0: t=0 z-gate x-part + biases
            xt0 = xsb.tile([128, KT, NB], mdt, name="x0", tag="xt")
            t0 = pdma(w_sb["wzx"][:], w_d["wzx"][:]) + pdma(xt0[:], xt_d[0])
            t0.append(nc.sync.dma_start(out=b_sb["bz"][:], in_=b_d["bz"][:]))
            t0.append(nc.sync.dma_start(out=b_sb["bh"][:], in_=b_d["bh"][:]))
            # Tier 1: t=0 g-gate x-part + t=1 x
            xt1 = xsb.tile([128, KT, NB], mdt, name="x1", tag="xt")
            t1 = pdma(w_sb["whx"][:], w_d["whx"][:]) + pdma(xt1[:], xt_d[1], pch=2)
            gate(t1, [t0[3], t0[7]])
            # Tier 2: t=1 x8/r weights + z h-part weights
            t2 = []
            if R8:
                xt8_1 = xsb.tile([128, 2, 2, NB], f8, name="x81", tag="xt8")
                t2.append(nc.sync.dma_start(out=xt8_1[:], in_=xt8_d[1]))
                t2 += w8dma("wrx8")
                if H8HALF:
                    t2 += w8dma("wzh8")
            else:
                t2 += pdma(w_sb["wrx"][:], w_d["wrx"][:])
            t2 += pdma(w_sb["wzh"][:], w_d["wzh"][:], pch=2)
            gate(t2, [t1[1], t1[3]])
            # Tier 3: t=1 recurrence tail
            t3 = []
            if R8:
                t3 += w8dma("wrh8")
                if H8HALF:
                    t3 += w8dma("whh8")
            else:
                t3 += pdma(w_sb["wrh"][:], w_d["wrh"][:])
            t3 += pdma(w_sb["whh"][:], w_d["whh"][:], pch=2)
            t3.append(nc.sync.dma_start(out=b_sb["br"][:], in_=b_d["br"][:]))
            gate(t3, [t2[-2], t2[0]])

            def wtile(wname, k, mi):
                off = (k * MT + mi) * 128
                return w_sb[wname][:, off:off + 128]

            h_prev = [None] * MT

            for t in range(T):
                if t == 0:
                    xt_t = xt0
                elif t == 1:
                    xt_t = xt1
                    if R8:
                        xt8_t = xt8_1
                else:
                    xt_t = xsb.tile([128, KT, NB], mdt, name=f"x{t}", tag="xt")
                    nc.sync.dma_start(out=xt_t[:], in_=xt_d[t])
                    if R8:
                        xt8_t = xsb.tile([128, 2, 2, NB], f8, name=f"x8{t}", tag="xt8")
                        nc.sync.dma_start(out=xt8_t[:], in_=xt8_d[t])

                def xts(k):
                    return xt_t[:, k, :]

                # --- PSUM: z and g as 2-bank tiles (one fused ACT each);
                # r as 2 single-bank tiles (split ACT so r0 lands early)
                zb = psum.tile([128, 1024], f32, name=f"z{t}", tag="zb")
                gb = psum.tile([128, 1024], f32, name=f"g{t}", tag="gb")
                if t > 0:
                    rb = [psum.tile([128, 512], f32, name=f"r{t}_{i}", tag=f"rb{i}") for i in range(2)]

                def half(bank, mi):
                    return bank[:, mi * NB:(mi + 1) * NB]

                def rhalf(mi):
                    return rb[mi // 2][:, (mi % 2) * NB:(mi % 2 + 1) * NB]

                # PSUM accumulation groups are tracked per BANK (zero
                # region): exactly one start=True (first MM into the bank)
                # and one stop=True (last MM into the bank) even though the
                # two m-halves are separate output regions.

                # --- x-projections (no recurrence dependency)
                if t == 0:
                    # k-outer so group k only needs the k-th wzx DMA chunk
                    for k in range(KT):
                        for mi in range(MT):
                            nc.tensor.matmul(half(zb, mi), wtile("wzx", k, mi), xts(k),
                                             start=(k == 0 and mi % 2 == 0),
                                             stop=(k == KT - 1 and mi % 2 == 1))
                    for k in range(KT):
                        for mi in range(MT):
                            nc.tensor.matmul(half(gb, mi), wtile("whx", k, mi), xts(k),
                                             start=(k == 0 and mi % 2 == 0),
                                             stop=(k == KT - 1 and mi % 2 == 1))
                else:
                    for mi in range(MT):
                        for k in range(KT):
                            nc.tensor.matmul(half(zb, mi), wtile("wzx", k, mi), xts(k),
                                             start=(mi % 2 == 0 and k == 0),
                                             stop=False)
                    if R8:
                        for k2 in range(2):
                            for mi in range(MT):
                                nc.tensor.matmul(rhalf(mi),
                                                 w8_sb["wrx8"][:, k2 * MT + mi, :, :],
                                                 xt8_t[:, k2],
                                                 start=(mi % 2 == 0 and k2 == 0),
                                                 stop=False, perf_mode=DR)
                    else:
                        for mi in range(MT):
                            for k in range(KT):
                                nc.tensor.matmul(rhalf(mi), wtile("wrx", k, mi), xts(k),
                                                 start=(mi % 2 == 0 and k == 0), stop=False)
                    for mi in range(MT):
                        for k in range(KT):
                            nc.tensor.matmul(half(gb, mi), wtile("whx", k, mi), xts(k),
                                             start=(mi % 2 == 0 and k == 0),
                                             stop=False)

                # --- recurrent parts
                if t > 0:
                    def hview(k):
                        return h_prev[k // 2][:, k % 2, :]

                    # z h-part: features 0..255 fp8-DR on h8, rest fp16
                    if H8HALF:
                        for mi in range(MT):
                            nc.tensor.matmul(half(zb, mi), w8_sb["wzh8"][:, mi, :, :],
                                             h8_prev[0][:], start=False, stop=False,
                                             perf_mode=DR)
                        for mi in range(MT):
                            for krel in range(2):
                                nc.tensor.matmul(half(zb, mi), wtile("wzh", krel, mi),
                                                 hview(krel + 2), start=False,
                                                 stop=(mi % 2 == 1 and krel == 1))
                    else:
                        for mi in range(MT):
                            for k in range(KT):
                                nc.tensor.matmul(half(zb, mi), wtile("wzh", k, mi), hview(k),
                                                 start=False,
                                                 stop=(mi % 2 == 1 and k == KT - 1))
                    if R8:
                        for k2 in range(2):
                            for mi in range(MT):
                                nc.tensor.matmul(rhalf(mi),
                                                 w8_sb["wrh8"][:, k2 * MT + mi, :, :],
                                                 h8_prev[k2][:],
                                                 start=False,
                                                 stop=(mi % 2 == 1 and k2 == 1),
                                                 perf_mode=DR)
                    else:
                        for mi in range(MT):
                            for k in range(KT):
                                nc.tensor.matmul(rhalf(mi), wtile("wrh", k, mi), hview(k),
                                                 start=False,
                                                 stop=(mi % 2 == 1 and k == KT - 1))

                    # r gate first (feeds r*h -> Whh matmuls)
                    r_t = []
                    for bi in range(2):
                        r_m = ssb.tile([128, 2, NB], mdt, name=f"r{t}b{bi}", tag=f"r{bi}")
                        if zero_bias:
                            nc.scalar.activation(
                                r_m[:], rb[bi][:].rearrange("p (m b) -> p m b", m=2), Sig,
                                scale=r_sc)
                        else:
                            for j in range(2):
                                nc.scalar.activation(
                                    r_m[:, j, :], rhalf(2 * bi + j), Sig,
                                    bias=b_sb["br"][:, 2 * bi + j:2 * bi + j + 1],
                                    scale=r_sc)
                        r_t.append(r_m)
                    # g h-part: features 0..255 fp8-DR on rh8a = fp8(r0*h0)
                    # (one fused mul+scale+cast on the Pool engine, available
                    # right after the early r0 sigmoid); features 256..511
                    # fp16 on rh1 from the DVE.
                    if H8HALF:
                        rh8a = ssb.tile([128, 2, NB], f8, name=f"rh8{t}", tag="rh8a")
                        nc.vector.scalar_tensor_tensor(
                            rh8a[:], r_t[0][:], SH, h_prev[0][:], Mult, Mult)
                        rh1 = ssb.tile([128, 2, NB], mdt, name=f"rh{t}b1", tag="rh1")
                        nc.vector.tensor_mul(rh1[:], r_t[1][:], h_prev[1][:])
                        for mi in range(MT):
                            nc.tensor.matmul(half(gb, mi), w8_sb["whh8"][:, mi, :, :],
                                             rh8a[:], start=False, stop=False,
                                             perf_mode=DR)
                        for mi in range(MT):
                            for krel in range(2):
                                nc.tensor.matmul(half(gb, mi), wtile("whh", krel, mi),
                                                 rh1[:, krel, :], start=False,
                                                 stop=(mi % 2 == 1 and krel == 1))
                    else:
                        rh0 = ssb.tile([128, 2, NB], mdt, name=f"rh{t}b0", tag="rh0")
                        nc.vector.tensor_mul(rh0[:], r_t[0][:], h_prev[0][:])
                        rh1 = ssb.tile([128, 2, NB], mdt, name=f"rh{t}b1", tag="rh1")
                        nc.vector.tensor_mul(rh1[:], r_t[1][:], h_prev[1][:])

                        def rhs_rh(k):
                            return (rh0 if k < 2 else rh1)[:, k % 2, :]

                        for mi in range(MT):
                            for k in range(KT):
                                nc.tensor.matmul(half(gb, mi), wtile("whh", k, mi), rhs_rh(k),
                                                 start=False,
                                                 stop=(mi % 2 == 1 and k == KT - 1))

                # --- gates and blend (z/g: one fused ACT over both banks)
                zm = ssb.tile([128, 4, NB], mdt, name=f"z{t}m", tag="z")
                gm = ssb.tile([128, 4, NB], mdt, name=f"g{t}m", tag="g")
                if zero_bias:
                    nc.scalar.activation(
                        zm[:], zb[:].rearrange("p (m b) -> p m b", m=4), Sig,
                        scale=zg_sc)
                    nc.scalar.activation(
                        gm[:], gb[:].rearrange("p (m b) -> p m b", m=4), Tanh,
                        scale=zg_sc)
                else:
                    for j in range(4):
                        nc.scalar.activation(
                            zm[:, j, :], half(zb, j), Sig,
                            bias=b_sb["bz"][:, j:j + 1], scale=zg_sc)
                    for j in range(4):
                        nc.scalar.activation(
                            gm[:, j, :], half(gb, j), Tanh,
                            bias=b_sb["bh"][:, j:j + 1], scale=zg_sc)
                z_t = [zm[:, 0:2, :], zm[:, 2:4, :]]
                g_t = [gm[:, 0:2, :], gm[:, 2:4, :]]

                h_t = []
                for bi in range(2):
                    hp = hsb.tile([128, 2, NB], mdt, name=f"h{t}p{bi}", tag=f"hp{bi}")
                    tmp = ssb.tile([128, 2, NB], mdt, name=f"tmp{t}b{bi}", tag=f"tmp{bi}")
                    if t == 0:
                        # h = (1 - z) * g = g - z*g
                        nc.vector.tensor_mul(tmp[:], z_t[bi], g_t[bi])
                        nc.vector.tensor_sub(hp[:], g_t[bi], tmp[:])
                    else:
                        # h = g + z*(h_prev - g)
                        nc.vector.tensor_sub(tmp[:], h_prev[bi][:], g_t[bi])
                        nc.vector.tensor_mul(tmp[:], tmp[:], z_t[bi])
                        nc.vector.tensor_add(hp[:], g_t[bi], tmp[:])
                    h_t.append(hp)
                    nc.sync.dma_start(out=out_d[t, :, bi], in_=hp[:])
                h_prev = h_t
                if R8 and t < T - 1:
                    h8_prev = []
                    for bi in range(2):
                        h8 = ssb.tile([128, 2, NB], f8, name=f"h8{t}b{bi}", tag=f"h8{bi}")
                        nc.scalar.activation(h8[:], h_t[bi][:], Copy, scale=SH)
                        h8_prev.append(h8)

    nc.compile()
    return nc


def _get_nc(zero_bias):
    key = (MM_DT, zero_bias, R8, H8HALF)
    if key not in _CACHE:
        _CACHE[key] = _build_nc(zero_bias)
    return _CACHE[key]


def _np_mdt():
    import ml_dtypes
    return {"fp16": np.float16, "bf16": ml_dtypes.bfloat16,
            "fp32r": np.float32}[MM_DT]


def _pack_w(W, ks=None, scale=1.0):
    # W [dout, din] -> lhsT tiles packed [128, len(ks)*MT*128], tile
    # (krel,m) at free offset (krel*MT+m)*128: w[p, off+q] = scale *
    # W[m*128+q, ks[krel]*128+p]
    if ks is None:
        ks = range(KT)
    Wt = (np.asarray(W, np.float32) * scale).T.reshape(KT, 128, MT, 128)[list(ks)]
    nk = Wt.shape[0]
    return np.ascontiguousarray(
        Wt.transpose(1, 0, 2, 3).reshape(128, nk * MT * 128)).astype(_np_mdt())


def _q8(a, scale):
    import ml_dtypes
    return np.clip(np.asarray(a, np.float32) * scale, -240, 240).astype(
        ml_dtypes.float8_e4m3)


def _pack_w8(W, scale, k2s=(0, 1)):
    # DoubleRow lhsT: w8[p, (k2rel mi i m)] = W[mi*128+m, (2*k2s[k2rel]+i)*128+p]
    # * scale
    Wt = np.asarray(W, np.float32).T.reshape(2, 2, 128, MT, 128)[list(k2s)]
    nk = Wt.shape[0]
    Wt = Wt.transpose(2, 0, 3, 1, 4).reshape(128, nk * MT * 2 * 128)
    return np.ascontiguousarray(_q8(Wt, scale))


def kernel(inputss, Wzx, Wzh, Wrx, Wrh, Whx, Whh, bz, br, bh):
    global LAST_RESULT
    from concourse.bass_utils import run_bass_kernel_spmd

    inputss = np.asarray(inputss, np.float32)
    assert inputss.shape == (N, T, D), inputss.shape

    zero_bias = (not np.any(np.asarray(bz)) and not np.any(np.asarray(br))
                 and not np.any(np.asarray(bh)))

    # host-side shard + swizzle to [NC, T, 128, KT, NB] (partition-major,
    # contiguous per partition row for fast DMA)
    x5 = inputss.reshape(NC, NB, T, KT, 128)           # (c, b, t, k, p)
    xs = np.ascontiguousarray(x5.transpose(0, 2, 4, 3, 1)).astype(_np_mdt())
    s16 = PROD if R8 else 1.0   # fp16 z/g weights carry the fp8 product scale
    wp = {"wzx": _pack_w(Wzx, scale=s16), "whx": _pack_w(Whx, scale=s16)}
    if H8HALF:
        wp["wzh"] = _pack_w(Wzh, ks=(2, 3), scale=s16)
        wp["whh"] = _pack_w(Whh, ks=(2, 3), scale=s16)
        wp["wzh8"] = _pack_w8(Wzh, SWH, k2s=(0,))
        wp["whh8"] = _pack_w8(Whh, SWH, k2s=(0,))
    else:
        wp["wzh"] = _pack_w(Wzh, scale=s16)
        wp["whh"] = _pack_w(Whh, scale=s16)
    if R8:
        wp["wrx8"] = _pack_w8(Wrx, SWX)
        wp["wrh8"] = _pack_w8(Wrh, SWH)
        # fp8 x, swizzled to [NC, T, 128, 2(k2), 2(i), NB]; k = 2*k2 + i
        x8_6 = inputss.reshape(NC, NB, T, 2, 2, 128)   # (c, b, t, k2, i, p)
        xs8 = _q8(np.ascontiguousarray(x8_6.transpose(0, 2, 5, 3, 4, 1)), SX)
    else:
        wp["wrx"] = _pack_w(Wrx)
        wp["wrh"] = _pack_w(Wrh)
    bp = {"bz": np.ascontiguousarray(np.asarray(bz, np.float32).reshape(MT, 128).T),
          "br": np.ascontiguousarray(np.asarray(br, np.float32).reshape(MT, 128).T),
          "bh": np.ascontiguousarray(np.asarray(bh, np.float32).reshape(MT, 128).T)}

    in_maps = []
    for c in range(NC):
        m = {"xt": np.ascontiguousarray(xs[c])}
        if R8:
            m["xt8"] = np.ascontiguousarray(xs8[c])
        m.update(wp)
        m.update(bp)
        in_maps.append(m)

    nc = _get_nc(zero_bias)
    trace = bool(int(os.environ.get("GRU_TRACE", "0")))
    res = run_bass_kernel_spmd(nc, in_maps, core_ids=list(range(NC)), trace=trace)
    LAST_RESULT = res

    # out: [NC, T, 128, 2(bi), 2(j), NB]; feature d = (2*bi+j)*128 + p
    outs = np.stack([np.asarray(res.results[c]["out"]) for c in range(NC)])
    return np.ascontiguousarray(
        outs.astype(np.float32).transpose(0, 5, 1, 3, 4, 2).reshape(N, T, D))
